# revision 21
# baseline (speedup 1.0000x reference)
"""BrokenBiasAttention Trainium2 kernel (8-core SPMD).

Sharding: core c -> batch b=c//2, query-row-half r=c%2 (1024 of 2048 rows).
Each core computes q for its rows, k/v for the whole batch, full 8-head
attention for its rows, and the output projection for its rows. Outputs are
disjoint row blocks -> gather is pure concatenation.

Device algorithm (per core):
  - all matmuls in bf16 (weights/x cast on host)
  - scores^T tiles [krow 128, qrow 512] via row-packed K=32 matmuls
  - softmax without max-subtraction (scores bounded ~|10|), constant shift 20:
      attn_un = exp(s - 20) * expF,   expF = exp(bias) gathered on device
  - bias is 3-level block-Toeplitz: host stages TW[h,rdw,w2,rh,w1] =
      T[h, 4r+rdw, rh, 15+w1-w2]  (pure replication / layout staging);
    device exps it once (small) and DMA-gathers 256-elem contiguous runs to
    build expF[h, rdw, half][128, 256] tiles in SBUF.
  - attn@v + rowsum via column-tiled matmuls accumulating in one PSUM bank
  - normalize: one DVE reciprocal per epilogue + DRAM-bounce broadcast
  - bias-multiply split between DVE and GpSimd.
"""

import math
import sys

import numpy as np

if "/opt/trn_rl_repo" not in sys.path:
    sys.path.insert(0, "/opt/trn_rl_repo")

N = 2048
C = 256
NH = 8
HD = 32
B = 4
QR = 1024  # q rows per core
S_SHIFT = 20.0

_NC = None


def _build_nc(dbg=False):
    import concourse.bass as bass
    import concourse.tile as tile
    from concourse import bacc, mybir
    from concourse.bass import ds, ts

    f32 = mybir.dt.float32
    bf16 = mybir.dt.bfloat16
    EXP = mybir.ActivationFunctionType.Exp

    nc = bacc.Bacc(None, target_bir_lowering=False, debug=False)

    xT = nc.dram_tensor("xT", [C, N], bf16, kind="ExternalInput")
    xTq = nc.dram_tensor("xTq", [C, QR], bf16, kind="ExternalInput")
    Wq_d = nc.dram_tensor("Wq", [C, C], bf16, kind="ExternalInput")
    Wk_d = nc.dram_tensor("Wk", [C, C], bf16, kind="ExternalInput")
    Wv_d = nc.dram_tensor("Wv", [C, C], bf16, kind="ExternalInput")
    Wo_d = nc.dram_tensor("Wo", [C, C], bf16, kind="ExternalInput")
    # TW[h, rdw(11), w2(16), rh(31), w1(16)]
    TW_d = nc.dram_tensor("TW", [NH, 11, 16, 31, 16], f32, kind="ExternalInput")
    out_d = nc.dram_tensor("out", [QR, C], f32, kind="ExternalOutput")

    assert 2 * 11 * 16 * 31 * 16 == 128 * 1364

    with tile.TileContext(nc) as tc:
        with (
            tc.tile_pool(name="consts", bufs=1) as consts,
            tc.tile_pool(name="twp", bufs=2) as twp,
            tc.tile_pool(name="etwp", bufs=2) as etwp,
            tc.tile_pool(name="expfp", bufs=1) as expfp,
            tc.tile_pool(name="xp", bufs=3) as xp,
            tc.tile_pool(name="kqv", bufs=1) as kqv,
            tc.tile_pool(name="ep", bufs=6) as ep,
            tc.tile_pool(name="rp", bufs=2) as rp,
            tc.tile_pool(name="otp", bufs=2) as otp,
            tc.tile_pool(name="stp", bufs=2) as stp,
            tc.tile_pool(name="spsum", bufs=3, space="PSUM") as spsum,
            tc.tile_pool(name="apsum", bufs=2, space="PSUM") as apsum,
            tc.tile_pool(name="dramp", bufs=4, space="DRAM") as dramp,
        ):
            # ---- expF construction: TW -> exp -> dram -> gather ----
            expf_sb = expfp.tile([128, NH * 11 * 384], bf16, tag="expf")
            expf_view = expf_sb.rearrange(
                "p (h r f) -> p h r f", h=NH, r=11, f=384
            )
            etw_d = dramp.tile([4, 128, 1364], bf16, name="etw_d")
            for hp in range(4):
                tw_sb = twp.tile([128, 1364], f32, tag="tw")
                src = TW_d[ds(2 * hp, 2)].rearrange(
                    "h r w2 rh w1 -> (h r w2 rh w1)"
                ).rearrange("(p f) -> p f", p=128)
                nc.scalar.dma_start(out=tw_sb, in_=src)
                etw_sb = etwp.tile([128, 1364], bf16, tag="etw")
                nc.scalar.activation(etw_sb, tw_sb, EXP)
                nc.scalar.dma_start(out=etw_d[hp], in_=etw_sb)
                # gather per h2': dest 16 partitions, free (2*rdw 22, 384)
                # union rh window rows 7-h2' .. 31-h2' (24 rows) covers both halves
                for h2p in range(8):
                    gap = bass.AP(
                        tensor=etw_d.tensor,
                        offset=etw_d.offset + hp * 174592 + (7 - h2p) * 16,
                        ap=[
                            [496, 16],    # w2 (partition)
                            [7936, 22],   # (h in pair, rdw) merged
                            [1, 384],     # (rh-window, w1) contiguous run
                        ],
                    )
                    geng = nc.gpsimd if h2p % 2 == 0 else nc.sync
                    geng.dma_start(
                        out=expf_view[ds(16 * h2p, 16), ds(2 * hp, 2)], in_=gap
                    )

            # ---- constants ----
            w_sb = {}
            for name, d in (("Wq", Wq_d), ("Wk", Wk_d), ("Wv", Wv_d), ("Wo", Wo_d)):
                t = consts.tile([128, 2, C], bf16, tag=f"w_{name}", name=f"w_{name}")
                nc.scalar.dma_start(out=t, in_=d[:].rearrange("(ch p) n -> p ch n", p=128))
                w_sb[name] = t
            ones_sb = consts.tile([128, 32], bf16, tag="ones")
            nc.vector.memset(ones_sb, 1.0)
            ebias = consts.tile([128, 1], f32, tag="ebias")
            nc.vector.memset(ebias, -S_SHIFT)

            if dbg:
                dbg_expf = nc.dram_tensor(
                    "dbg_expf", [128, NH * 11 * 384], bf16,
                    kind="ExternalOutput")
                nc.sync.dma_start(out=dbg_expf[:], in_=expf_sb)

            # ---- projections (all bf16) ----
            kT_sb = [kqv.tile([128, N], bf16, tag=f"kT{m}", name=f"kT{m}")
                     for m in range(2)]
            qT_sb = [kqv.tile([128, QR], bf16, tag=f"qT{m}", name=f"qT{m}")
                     for m in range(2)]
            v_sb = kqv.tile([128, 16, C], bf16, tag="v")
            qscale = 1.0 / math.sqrt(HD)

            xTq_r = xTq[:].rearrange("(ch p) n -> p ch n", p=128)
            for j in range(QR // 512):
                xq = xp.tile([128, 2, 512], bf16, tag="x")
                nc.scalar.dma_start(out=xq, in_=xTq_r[:, :, ds(512 * j, 512)])
                for m in range(2):
                    ps = spsum.tile([128, 1024], f32, tag="s")
                    for ch in range(2):
                        nc.tensor.matmul(
                            ps[:, :512],
                            lhsT=w_sb["Wq"][:, ch, ts(m, 128)],
                            rhs=xq[:, ch, :],
                            start=(ch == 0),
                            stop=(ch == 1),
                        )
                    nc.vector.tensor_scalar_mul(
                        qT_sb[m][:, ds(512 * j, 512)], ps[:, :512], qscale
                    )

            xT_r = xT[:].rearrange("(ch p) n -> p ch n", p=128)
            for j in range(N // 512):
                xc = xp.tile([128, 2, 512], bf16, tag="x")
                nc.scalar.dma_start(out=xc, in_=xT_r[:, :, ds(512 * j, 512)])
                for m in range(2):
                    ps = spsum.tile([128, 1024], f32, tag="s")
                    for ch in range(2):
                        nc.tensor.matmul(
                            ps[:, :512],
                            lhsT=w_sb["Wk"][:, ch, ts(m, 128)],
                            rhs=xc[:, ch, :],
                            start=(ch == 0),
                            stop=(ch == 1),
                        )
                    nc.vector.tensor_copy(kT_sb[m][:, ds(512 * j, 512)], ps[:, :512])
                for t in range(4):
                    kt = 4 * j + t
                    ps = spsum.tile([128, 1024], f32, tag="s")
                    for ch in range(2):
                        nc.tensor.matmul(
                            ps[:, :C],
                            lhsT=xc[:, ch, ts(t, 128)],
                            rhs=w_sb["Wv"][:, ch, :],
                            start=(ch == 0),
                            stop=(ch == 1),
                        )
                    nc.vector.tensor_copy(v_sb[:, kt, :], ps[:, :C])

            # ---- main attention loops ----
            oT_tiles = []
            for qc in range(2):
                oT = otp.tile([128, 2, 512], bf16, tag="oT", name=f"oT{qc}")
                oT_tiles.append(oT)
            for g2 in range(4):
                for qc in range(2):
                    oT = oT_tiles[qc]
                    po_av = 0 if g2 % 2 == 0 else 64
                    po_rs = 64 - po_av
                    half_idx = g2 // 2
                    acc = apsum.tile([128, 512], f32, tag="acc")
                    for kt in range(16):
                        s_ps = spsum.tile([128, 1024], f32, tag="s")
                        for k in range(2):
                            h = 2 * g2 + k
                            i = h % 4
                            nc.tensor.matmul(
                                s_ps[:, ts(k, 512)],
                                lhsT=kT_sb[half_idx][ds(32 * i, 32), ts(kt, 128)],
                                rhs=qT_sb[half_idx][ds(32 * i, 32), ts(qc, 512)],
                                start=True,
                                stop=True,
                                tile_position=(32 * i, 0),
                            )
                        e_sb = ep.tile([128, 1024], bf16, tag="e")
                        nc.scalar.activation(e_sb, s_ps, EXP, bias=ebias[:, :])
                        rdw0 = 2 * qc - (kt // 2) + 7
                        woff = 128 if kt % 2 == 0 else 0
                        ev = e_sb.rearrange("p (k jj f) -> p k jj f", k=2, jj=2)
                        fv = expf_view[
                            :, ds(2 * g2, 2), ds(rdw0, 2), ds(woff, 256)
                        ]
                        nc.vector.tensor_mul(ev, ev, fv)
                        for k in range(2):
                            h = 2 * g2 + k
                            nc.tensor.matmul(
                                acc[ds(po_av + 32 * k, 32), :],
                                lhsT=v_sb[:, kt, ds(32 * h, 32)],
                                rhs=e_sb[:, ts(k, 512)],
                                start=(kt == 0),
                                stop=(kt == 15),
                                tile_position=(0, po_av + 32 * k),
                                skip_group_check=True,
                            )
                            nc.tensor.matmul(
                                acc[ds(po_rs + 32 * k, 32), :],
                                lhsT=ones_sb,
                                rhs=e_sb[:, ts(k, 512)],
                                start=(kt == 0),
                                stop=(kt == 15),
                                tile_position=(0, po_rs + 32 * k),
                                skip_group_check=True,
                            )
                    # epilogue: normalize 2 heads into oT
                    recip = rp.tile([128, 512], f32, tag="recip")
                    rep = rp.tile([128, 512], f32, tag="rep")
                    nc.vector.tensor_copy(
                        recip[ds(po_rs, 64), :], acc[ds(po_rs, 64), :]
                    )
                    nc.vector.reciprocal(
                        recip[ds(po_rs, 64), :], recip[ds(po_rs, 64), :]
                    )
                    nc.sync.dma_start(
                        out=rep[ds(po_av, 64), :], in_=recip[ds(po_rs, 64), :]
                    )
                    nc.vector.tensor_mul(
                        oT[ds(po_av, 64), half_idx, :],
                        acc[ds(po_av, 64), :],
                        rep[ds(po_av, 64), :],
                    )
            # final projections (after both qc loops; off the loop critical path)
            for qc in range(2):
                oT = oT_tiles[qc]
                for s in range(4):
                    fps = spsum.tile([128, 1024], f32, tag="s")
                    for ch in range(2):
                        nc.tensor.matmul(
                            fps[:, :C],
                            lhsT=oT[:, ch, ts(s, 128)],
                            rhs=w_sb["Wo"][:, ch, :],
                            start=(ch == 0),
                            stop=(ch == 1),
                        )
                    stage = stp.tile([128, C], f32, tag="stage")
                    nc.vector.tensor_copy(stage, fps[:, :C])
                    nc.sync.dma_start(
                        out=out_d[ds(512 * qc + 128 * s, 128), :], in_=stage
                    )

    nc.compile()
    return nc


def _host_inputs(x, Wq, Wk, Wv, Wo, bias_table):
    """Build the 8 per-core input maps."""
    import ml_dtypes

    bf = ml_dtypes.bfloat16
    x = np.asarray(x, dtype=np.float32)
    T = np.asarray(bias_table, dtype=np.float32)
    xf = np.ascontiguousarray(x.reshape(B, N, C))
    idx_w = 15 + np.arange(16)[None, :] - np.arange(16)[:, None]  # [w2, w1]
    Ws = {
        "Wq": np.ascontiguousarray(np.asarray(Wq, np.float32).astype(bf)),
        "Wk": np.ascontiguousarray(np.asarray(Wk, np.float32).astype(bf)),
        "Wv": np.ascontiguousarray(np.asarray(Wv, np.float32).astype(bf)),
        "Wo": np.ascontiguousarray(np.asarray(Wo, np.float32).astype(bf)),
    }
    in_maps = []
    for c in range(8):
        b, r = c // 2, c % 2
        d1min = 4 * r
        Twin = T[:, d1min:d1min + 11]                     # [8, 11, 31, 31]
        TW = Twin[:, :, :, idx_w]                         # [8,11,31,16,16] (h,rdw,rh,w2,w1)
        TW = np.ascontiguousarray(TW.transpose(0, 1, 3, 2, 4))  # [h,rdw,w2,rh,w1]
        in_maps.append({
            "xT": np.ascontiguousarray(xf[b].T.astype(bf)),
            "xTq": np.ascontiguousarray(xf[b, QR * r:QR * (r + 1)].T.astype(bf)),
            "TW": TW,
            **Ws,
        })
    return in_maps


def kernel(x, Wq, Wk, Wv, Wo, bias_table, _results_hook=None):
    global _NC
    if _NC is None:
        _NC = _build_nc()
    from concourse.bass_utils import run_bass_kernel_spmd

    in_maps = _host_inputs(x, Wq, Wk, Wv, Wo, bias_table)
    res = run_bass_kernel_spmd(_NC, in_maps, core_ids=list(range(8)))
    if _results_hook is not None:
        _results_hook(res)
    out = np.zeros((B, N, C), dtype=np.float32)
    for c in range(8):
        b, r = c // 2, c % 2
        out[b, QR * r:QR * (r + 1)] = res.results[c]["out"]
    D, H, W = 8, 16, 16
    return out.reshape(B, D, H, W, C)


# revision 22
# speedup vs baseline: 1.0485x; 1.0485x over previous
"""BrokenBiasAttention Trainium2 kernel (8-core SPMD).

Sharding: core c -> batch b=c//2, query-row-half r=c%2 (1024 of 2048 rows).
Each core computes q for its rows, k/v for the whole batch, full 8-head
attention for its rows, and the output projection for its rows. Outputs are
disjoint row blocks -> gather is pure concatenation.

Device algorithm (per core):
  - all matmuls in bf16 (weights/x cast on host)
  - scores^T tiles [krow 128, qrow 512] via row-packed K=32 matmuls
  - softmax without max-subtraction (scores bounded ~|10|), constant shift 20:
      attn_un = exp(s - 20) * expF,   expF = exp(bias) gathered on device
  - bias is 3-level block-Toeplitz: host stages TW[h,rdw,w2,rh,w1] =
      T[h, 4r+rdw, rh, 15+w1-w2]  (pure replication / layout staging);
    device exps it once (small) and DMA-gathers 256-elem contiguous runs to
    build expF[h, rdw, half][128, 256] tiles in SBUF.
  - attn@v + rowsum via column-tiled matmuls accumulating in one PSUM bank
  - normalize: one DVE reciprocal per epilogue + DRAM-bounce broadcast
  - bias-multiply split between DVE and GpSimd.
"""

import math
import sys

import numpy as np

if "/opt/trn_rl_repo" not in sys.path:
    sys.path.insert(0, "/opt/trn_rl_repo")

N = 2048
C = 256
NH = 8
HD = 32
B = 4
QR = 1024  # q rows per core
S_SHIFT = 20.0

_NC = None


def _build_nc(dbg=False):
    import concourse.bass as bass
    import concourse.tile as tile
    from concourse import bacc, mybir
    from concourse.bass import ds, ts

    f32 = mybir.dt.float32
    bf16 = mybir.dt.bfloat16
    EXP = mybir.ActivationFunctionType.Exp

    nc = bacc.Bacc(None, target_bir_lowering=False, debug=False)

    xT = nc.dram_tensor("xT", [C, N], bf16, kind="ExternalInput")
    xTq = nc.dram_tensor("xTq", [C, QR], bf16, kind="ExternalInput")
    Wq_d = nc.dram_tensor("Wq", [C, C], bf16, kind="ExternalInput")
    Wk_d = nc.dram_tensor("Wk", [C, C], bf16, kind="ExternalInput")
    Wv_d = nc.dram_tensor("Wv", [C, C], bf16, kind="ExternalInput")
    Wo_d = nc.dram_tensor("Wo", [C, C], bf16, kind="ExternalInput")
    # TW[h, rdw(11), w2(16), rh(31), w1(16)]
    TW_d = nc.dram_tensor("TW", [NH, 11, 16, 31, 16], f32, kind="ExternalInput")
    out_d = nc.dram_tensor("out", [QR, C], f32, kind="ExternalOutput")

    assert 2 * 11 * 16 * 31 * 16 == 128 * 1364

    with tile.TileContext(nc) as tc:
        with (
            tc.tile_pool(name="consts", bufs=1) as consts,
            tc.tile_pool(name="twp", bufs=2) as twp,
            tc.tile_pool(name="etwp", bufs=2) as etwp,
            tc.tile_pool(name="expfp", bufs=1) as expfp,
            tc.tile_pool(name="xp", bufs=3) as xp,
            tc.tile_pool(name="kqv", bufs=1) as kqv,
            tc.tile_pool(name="ep", bufs=6) as ep,
            tc.tile_pool(name="rp", bufs=2) as rp,
            tc.tile_pool(name="otp", bufs=2) as otp,
            tc.tile_pool(name="stp", bufs=2) as stp,
            tc.tile_pool(name="spsum", bufs=3, space="PSUM") as spsum,
            tc.tile_pool(name="apsum", bufs=2, space="PSUM") as apsum,
            tc.tile_pool(name="dramp", bufs=4, space="DRAM") as dramp,
        ):
            # ---- expF construction: TW -> exp -> dram -> gather ----
            expf_sb = expfp.tile([128, NH * 11 * 384], bf16, tag="expf")
            expf_view = expf_sb.rearrange(
                "p (h r f) -> p h r f", h=NH, r=11, f=384
            )
            etw_d = dramp.tile([4, 128, 1364], bf16, name="etw_d")
            for hp in range(4):
                tw_sb = twp.tile([128, 1364], f32, tag="tw")
                src = TW_d[ds(2 * hp, 2)].rearrange(
                    "h r w2 rh w1 -> (h r w2 rh w1)"
                ).rearrange("(p f) -> p f", p=128)
                nc.scalar.dma_start(out=tw_sb, in_=src)
                etw_sb = etwp.tile([128, 1364], bf16, tag="etw")
                nc.scalar.activation(etw_sb, tw_sb, EXP)
                nc.scalar.dma_start(out=etw_d[hp], in_=etw_sb)
                # gather per h2': dest 16 partitions, free (2*rdw 22, 384)
                # union rh window rows 7-h2' .. 31-h2' (24 rows) covers both halves
                for h2p in range(8):
                    gap = bass.AP(
                        tensor=etw_d.tensor,
                        offset=etw_d.offset + hp * 174592 + (7 - h2p) * 16,
                        ap=[
                            [496, 16],    # w2 (partition)
                            [7936, 22],   # (h in pair, rdw) merged
                            [1, 384],     # (rh-window, w1) contiguous run
                        ],
                    )
                    geng = nc.gpsimd if h2p % 2 == 0 else nc.sync
                    geng.dma_start(
                        out=expf_view[ds(16 * h2p, 16), ds(2 * hp, 2)], in_=gap
                    )

            # ---- constants ----
            w_sb = {}
            for name, d in (("Wq", Wq_d), ("Wk", Wk_d), ("Wv", Wv_d), ("Wo", Wo_d)):
                t = consts.tile([128, 2, C], bf16, tag=f"w_{name}", name=f"w_{name}")
                nc.sync.dma_start(out=t, in_=d[:].rearrange("(ch p) n -> p ch n", p=128))
                w_sb[name] = t
            ones_sb = consts.tile([128, 32], bf16, tag="ones")
            nc.vector.memset(ones_sb, 1.0)
            ebias = consts.tile([128, 1], f32, tag="ebias")
            nc.vector.memset(ebias, -S_SHIFT)

            if dbg:
                dbg_expf = nc.dram_tensor(
                    "dbg_expf", [128, NH * 11 * 384], bf16,
                    kind="ExternalOutput")
                nc.sync.dma_start(out=dbg_expf[:], in_=expf_sb)

            # ---- projections (all bf16) ----
            kT_sb = [kqv.tile([128, N], bf16, tag=f"kT{m}", name=f"kT{m}")
                     for m in range(2)]
            qT_sb = [kqv.tile([128, QR], bf16, tag=f"qT{m}", name=f"qT{m}")
                     for m in range(2)]
            v_sb = kqv.tile([128, 16, C], bf16, tag="v")
            qscale = 1.0 / math.sqrt(HD)

            xTq_r = xTq[:].rearrange("(ch p) n -> p ch n", p=128)
            for j in range(QR // 512):
                xq = xp.tile([128, 2, 512], bf16, tag="x")
                nc.sync.dma_start(out=xq, in_=xTq_r[:, :, ds(512 * j, 512)])
                for m in range(2):
                    ps = spsum.tile([128, 1024], f32, tag="s")
                    for ch in range(2):
                        nc.tensor.matmul(
                            ps[:, :512],
                            lhsT=w_sb["Wq"][:, ch, ts(m, 128)],
                            rhs=xq[:, ch, :],
                            start=(ch == 0),
                            stop=(ch == 1),
                        )
                    nc.vector.tensor_scalar_mul(
                        qT_sb[m][:, ds(512 * j, 512)], ps[:, :512], qscale
                    )

            xT_r = xT[:].rearrange("(ch p) n -> p ch n", p=128)
            for j in range(N // 512):
                xc = xp.tile([128, 2, 512], bf16, tag="x")
                nc.sync.dma_start(out=xc, in_=xT_r[:, :, ds(512 * j, 512)])
                for m in range(2):
                    ps = spsum.tile([128, 1024], f32, tag="s")
                    for ch in range(2):
                        nc.tensor.matmul(
                            ps[:, :512],
                            lhsT=w_sb["Wk"][:, ch, ts(m, 128)],
                            rhs=xc[:, ch, :],
                            start=(ch == 0),
                            stop=(ch == 1),
                        )
                    nc.vector.tensor_copy(kT_sb[m][:, ds(512 * j, 512)], ps[:, :512])
                for t in range(4):
                    kt = 4 * j + t
                    ps = spsum.tile([128, 1024], f32, tag="s")
                    for ch in range(2):
                        nc.tensor.matmul(
                            ps[:, :C],
                            lhsT=xc[:, ch, ts(t, 128)],
                            rhs=w_sb["Wv"][:, ch, :],
                            start=(ch == 0),
                            stop=(ch == 1),
                        )
                    nc.vector.tensor_copy(v_sb[:, kt, :], ps[:, :C])

            # ---- main attention loops ----
            oT_tiles = []
            for qc in range(2):
                oT = otp.tile([128, 2, 512], bf16, tag="oT", name=f"oT{qc}")
                oT_tiles.append(oT)
            for g2 in range(4):
                for qc in range(2):
                    oT = oT_tiles[qc]
                    po_av = 0 if g2 % 2 == 0 else 64
                    po_rs = 64 - po_av
                    half_idx = g2 // 2
                    acc = apsum.tile([128, 512], f32, tag="acc")
                    for kt in range(16):
                        s_ps = spsum.tile([128, 1024], f32, tag="s")
                        for k in range(2):
                            h = 2 * g2 + k
                            i = h % 4
                            nc.tensor.matmul(
                                s_ps[:, ts(k, 512)],
                                lhsT=kT_sb[half_idx][ds(32 * i, 32), ts(kt, 128)],
                                rhs=qT_sb[half_idx][ds(32 * i, 32), ts(qc, 512)],
                                start=True,
                                stop=True,
                                tile_position=(32 * i, 0),
                            )
                        e_sb = ep.tile([128, 1024], bf16, tag="e")
                        nc.scalar.activation(e_sb, s_ps, EXP, bias=ebias[:, :])
                        rdw0 = 2 * qc - (kt // 2) + 7
                        woff = 128 if kt % 2 == 0 else 0
                        ev = e_sb.rearrange("p (k jj f) -> p k jj f", k=2, jj=2)
                        fv = expf_view[
                            :, ds(2 * g2, 2), ds(rdw0, 2), ds(woff, 256)
                        ]
                        nc.vector.tensor_mul(ev, ev, fv)
                        for k in range(2):
                            h = 2 * g2 + k
                            nc.tensor.matmul(
                                acc[ds(po_av + 32 * k, 32), :],
                                lhsT=v_sb[:, kt, ds(32 * h, 32)],
                                rhs=e_sb[:, ts(k, 512)],
                                start=(kt == 0),
                                stop=(kt == 15),
                                tile_position=(0, po_av + 32 * k),
                                skip_group_check=True,
                            )
                            nc.tensor.matmul(
                                acc[ds(po_rs + 32 * k, 32), :],
                                lhsT=ones_sb,
                                rhs=e_sb[:, ts(k, 512)],
                                start=(kt == 0),
                                stop=(kt == 15),
                                tile_position=(0, po_rs + 32 * k),
                                skip_group_check=True,
                            )
                    # epilogue: normalize 2 heads into oT
                    recip = rp.tile([128, 512], f32, tag="recip")
                    rep = rp.tile([128, 512], f32, tag="rep")
                    nc.vector.tensor_copy(
                        recip[ds(po_rs, 64), :], acc[ds(po_rs, 64), :]
                    )
                    nc.vector.reciprocal(
                        recip[ds(po_rs, 64), :], recip[ds(po_rs, 64), :]
                    )
                    nc.sync.dma_start(
                        out=rep[ds(po_av, 64), :], in_=recip[ds(po_rs, 64), :]
                    )
                    nc.vector.tensor_mul(
                        oT[ds(po_av, 64), half_idx, :],
                        acc[ds(po_av, 64), :],
                        rep[ds(po_av, 64), :],
                    )
            # final projections (after both qc loops; off the loop critical path)
            for qc in range(2):
                oT = oT_tiles[qc]
                for s in range(4):
                    fps = spsum.tile([128, 1024], f32, tag="s")
                    for ch in range(2):
                        nc.tensor.matmul(
                            fps[:, :C],
                            lhsT=oT[:, ch, ts(s, 128)],
                            rhs=w_sb["Wo"][:, ch, :],
                            start=(ch == 0),
                            stop=(ch == 1),
                        )
                    stage = stp.tile([128, C], f32, tag="stage")
                    nc.vector.tensor_copy(stage, fps[:, :C])
                    nc.sync.dma_start(
                        out=out_d[ds(512 * qc + 128 * s, 128), :], in_=stage
                    )

    nc.compile()
    return nc


def _host_inputs(x, Wq, Wk, Wv, Wo, bias_table):
    """Build the 8 per-core input maps."""
    import ml_dtypes

    bf = ml_dtypes.bfloat16
    x = np.asarray(x, dtype=np.float32)
    T = np.asarray(bias_table, dtype=np.float32)
    xf = np.ascontiguousarray(x.reshape(B, N, C))
    idx_w = 15 + np.arange(16)[None, :] - np.arange(16)[:, None]  # [w2, w1]
    Ws = {
        "Wq": np.ascontiguousarray(np.asarray(Wq, np.float32).astype(bf)),
        "Wk": np.ascontiguousarray(np.asarray(Wk, np.float32).astype(bf)),
        "Wv": np.ascontiguousarray(np.asarray(Wv, np.float32).astype(bf)),
        "Wo": np.ascontiguousarray(np.asarray(Wo, np.float32).astype(bf)),
    }
    in_maps = []
    for c in range(8):
        b, r = c // 2, c % 2
        d1min = 4 * r
        Twin = T[:, d1min:d1min + 11]                     # [8, 11, 31, 31]
        TW = Twin[:, :, :, idx_w]                         # [8,11,31,16,16] (h,rdw,rh,w2,w1)
        TW = np.ascontiguousarray(TW.transpose(0, 1, 3, 2, 4))  # [h,rdw,w2,rh,w1]
        in_maps.append({
            "xT": np.ascontiguousarray(xf[b].T.astype(bf)),
            "xTq": np.ascontiguousarray(xf[b, QR * r:QR * (r + 1)].T.astype(bf)),
            "TW": TW,
            **Ws,
        })
    return in_maps


def kernel(x, Wq, Wk, Wv, Wo, bias_table, _results_hook=None):
    global _NC
    if _NC is None:
        _NC = _build_nc()
    from concourse.bass_utils import run_bass_kernel_spmd

    in_maps = _host_inputs(x, Wq, Wk, Wv, Wo, bias_table)
    res = run_bass_kernel_spmd(_NC, in_maps, core_ids=list(range(8)))
    if _results_hook is not None:
        _results_hook(res)
    out = np.zeros((B, N, C), dtype=np.float32)
    for c in range(8):
        b, r = c // 2, c % 2
        out[b, QR * r:QR * (r + 1)] = res.results[c]["out"]
    D, H, W = 8, 16, 16
    return out.reshape(B, D, H, W, C)


# revision 23
# speedup vs baseline: 1.1163x; 1.0647x over previous
"""BrokenBiasAttention Trainium2 kernel (8-core SPMD).

Sharding: core c -> batch b=c//2, query-row-half r=c%2 (1024 of 2048 rows).
Each core computes q for its rows, k/v for the whole batch, full 8-head
attention for its rows, and the output projection for its rows. Outputs are
disjoint row blocks -> gather is pure concatenation.

Device algorithm (per core):
  - all matmuls in bf16 (weights/x cast on host)
  - scores^T tiles [krow 128, qrow 512] via row-packed K=32 matmuls
  - softmax without max-subtraction (scores bounded ~|10|), constant shift 20:
      attn_un = exp(s - 20) * expF,   expF = exp(bias) gathered on device
  - bias is 3-level block-Toeplitz: host stages TW[h,rdw,w2,rh,w1] =
      T[h, 4r+rdw, rh, 15+w1-w2]  (pure replication / layout staging);
    device exps it once (small) and DMA-gathers 256-elem contiguous runs to
    build expF[h, rdw, half][128, 256] tiles in SBUF.
  - attn@v + rowsum via column-tiled matmuls accumulating in one PSUM bank
  - normalize: one DVE reciprocal per epilogue + DRAM-bounce broadcast
  - bias-multiply split between DVE and GpSimd.
"""

import math
import sys

import numpy as np

if "/opt/trn_rl_repo" not in sys.path:
    sys.path.insert(0, "/opt/trn_rl_repo")

N = 2048
C = 256
NH = 8
HD = 32
B = 4
QR = 1024  # q rows per core
S_SHIFT = 20.0

_NC = None


def _build_nc(dbg=False):
    import concourse.bass as bass
    import concourse.tile as tile
    from concourse import bacc, mybir
    from concourse.bass import ds, ts

    f32 = mybir.dt.float32
    bf16 = mybir.dt.bfloat16
    EXP = mybir.ActivationFunctionType.Exp

    nc = bacc.Bacc(None, target_bir_lowering=False, debug=False)

    xT = nc.dram_tensor("xT", [C, N], bf16, kind="ExternalInput")
    xTq = nc.dram_tensor("xTq", [C, QR], bf16, kind="ExternalInput")
    Wq_d = nc.dram_tensor("Wq", [C, C], bf16, kind="ExternalInput")
    Wk_d = nc.dram_tensor("Wk", [C, C], bf16, kind="ExternalInput")
    Wv_d = nc.dram_tensor("Wv", [C, C], bf16, kind="ExternalInput")
    Wo_d = nc.dram_tensor("Wo", [C, C], bf16, kind="ExternalInput")
    # TW[h, rdw(11), w2(16), rh(31), w1(16)]
    TW_d = nc.dram_tensor("TW", [NH, 11, 16, 31, 16], f32, kind="ExternalInput")
    out_d = nc.dram_tensor("out", [QR, C], f32, kind="ExternalOutput")

    assert 2 * 11 * 16 * 31 * 16 == 128 * 1364

    with tile.TileContext(nc) as tc:
        with (
            tc.tile_pool(name="consts", bufs=1) as consts,
            tc.tile_pool(name="twp", bufs=2) as twp,
            tc.tile_pool(name="etwp", bufs=2) as etwp,
            tc.tile_pool(name="expfp", bufs=1) as expfp,
            tc.tile_pool(name="xp", bufs=3) as xp,
            tc.tile_pool(name="kqv", bufs=1) as kqv,
            tc.tile_pool(name="ep", bufs=6) as ep,
            tc.tile_pool(name="rp", bufs=2) as rp,
            tc.tile_pool(name="otp", bufs=2) as otp,
            tc.tile_pool(name="stp", bufs=2) as stp,
            tc.tile_pool(name="spsum", bufs=3, space="PSUM") as spsum,
            tc.tile_pool(name="apsum", bufs=2, space="PSUM") as apsum,
            tc.tile_pool(name="dramp", bufs=4, space="DRAM") as dramp,
        ):
            # ---- expF construction: TW -> exp -> dram -> gather ----
            expf_sb = expfp.tile([128, NH * 11 * 384], bf16, tag="expf")
            expf_view = expf_sb.rearrange(
                "p (h r f) -> p h r f", h=NH, r=11, f=384
            )
            etw_d = dramp.tile([4, 128, 1364], bf16, name="etw_d")
            for hp in range(4):
                tw_sb = twp.tile([128, 1364], f32, tag="tw")
                src = TW_d[ds(2 * hp, 2)].rearrange(
                    "h r w2 rh w1 -> (h r w2 rh w1)"
                ).rearrange("(p f) -> p f", p=128)
                nc.scalar.dma_start(out=tw_sb, in_=src)
                etw_sb = etwp.tile([128, 1364], bf16, tag="etw")
                nc.scalar.activation(etw_sb, tw_sb, EXP)
                nc.scalar.dma_start(out=etw_d[hp], in_=etw_sb)
                # gather per h2': dest 16 partitions, free (2*rdw 22, 384)
                # union rh window rows 7-h2' .. 31-h2' (24 rows) covers both halves
                for h2p in range(8):
                    gap = bass.AP(
                        tensor=etw_d.tensor,
                        offset=etw_d.offset + hp * 174592 + (7 - h2p) * 16,
                        ap=[
                            [496, 16],    # w2 (partition)
                            [7936, 22],   # (h in pair, rdw) merged
                            [1, 384],     # (rh-window, w1) contiguous run
                        ],
                    )
                    geng = nc.gpsimd if h2p % 2 == 0 else nc.sync
                    geng.dma_start(
                        out=expf_view[ds(16 * h2p, 16), ds(2 * hp, 2)], in_=gap
                    )

            # ---- constants ----
            w_sb = {}
            for name, d in (("Wq", Wq_d), ("Wk", Wk_d), ("Wv", Wv_d), ("Wo", Wo_d)):
                t = consts.tile([128, 2, C], bf16, tag=f"w_{name}", name=f"w_{name}")
                nc.sync.dma_start(out=t, in_=d[:].rearrange("(ch p) n -> p ch n", p=128))
                w_sb[name] = t
            ones_sb = consts.tile([128, 32], bf16, tag="ones")
            nc.vector.memset(ones_sb, 1.0)
            ebias = consts.tile([128, 1], f32, tag="ebias")
            nc.vector.memset(ebias, -S_SHIFT)

            if dbg:
                dbg_expf = nc.dram_tensor(
                    "dbg_expf", [128, NH * 11 * 384], bf16,
                    kind="ExternalOutput")
                nc.sync.dma_start(out=dbg_expf[:], in_=expf_sb)

            # ---- projections (all bf16) ----
            kT_sb = [kqv.tile([128, N], bf16, tag=f"kT{m}", name=f"kT{m}")
                     for m in range(2)]
            qT_sb = [kqv.tile([128, QR], bf16, tag=f"qT{m}", name=f"qT{m}")
                     for m in range(2)]
            v_sb = kqv.tile([128, 16, C], bf16, tag="v")
            qscale = 1.0 / math.sqrt(HD)

            xTq_r = xTq[:].rearrange("(ch p) n -> p ch n", p=128)
            for j in range(QR // 512):
                xq = xp.tile([128, 2, 512], bf16, tag="x")
                nc.sync.dma_start(out=xq, in_=xTq_r[:, :, ds(512 * j, 512)])
                for m in range(2):
                    ps = spsum.tile([128, 1024], f32, tag="s")
                    for ch in range(2):
                        nc.tensor.matmul(
                            ps[:, :512],
                            lhsT=w_sb["Wq"][:, ch, ts(m, 128)],
                            rhs=xq[:, ch, :],
                            start=(ch == 0),
                            stop=(ch == 1),
                        )
                    nc.vector.tensor_scalar_mul(
                        qT_sb[m][:, ds(512 * j, 512)], ps[:, :512], qscale
                    )

            xT_r = xT[:].rearrange("(ch p) n -> p ch n", p=128)
            for j in range(N // 512):
                xc = xp.tile([128, 2, 512], bf16, tag="x")
                nc.sync.dma_start(out=xc, in_=xT_r[:, :, ds(512 * j, 512)])
                for m in range(2):
                    ps = spsum.tile([128, 1024], f32, tag="s")
                    for ch in range(2):
                        nc.tensor.matmul(
                            ps[:, :512],
                            lhsT=w_sb["Wk"][:, ch, ts(m, 128)],
                            rhs=xc[:, ch, :],
                            start=(ch == 0),
                            stop=(ch == 1),
                        )
                    nc.vector.tensor_copy(kT_sb[m][:, ds(512 * j, 512)], ps[:, :512])
                for t in range(4):
                    kt = 4 * j + t
                    ps = spsum.tile([128, 1024], f32, tag="s")
                    for ch in range(2):
                        nc.tensor.matmul(
                            ps[:, :C],
                            lhsT=xc[:, ch, ts(t, 128)],
                            rhs=w_sb["Wv"][:, ch, :],
                            start=(ch == 0),
                            stop=(ch == 1),
                        )
                    nc.vector.tensor_copy(v_sb[:, kt, :], ps[:, :C])

            # ---- main attention loops ----
            oT_tiles = []
            for qc in range(2):
                oT = otp.tile([128, 2, 512], bf16, tag="oT", name=f"oT{qc}")
                oT_tiles.append(oT)
            for g2 in range(4):
                for qc in range(2):
                    oT = oT_tiles[qc]
                    po_av = 0 if g2 % 2 == 0 else 64
                    po_rs = 64 - po_av
                    half_idx = g2 // 2
                    acc = apsum.tile([128, 512], f32, tag="acc")
                    e_tiles = {}

                    def emit_av(kt):
                        e_t = e_tiles.pop(kt)
                        for k in range(2):
                            h = 2 * g2 + k
                            nc.tensor.matmul(
                                acc[ds(po_av + 32 * k, 32), :],
                                lhsT=v_sb[:, kt, ds(32 * h, 32)],
                                rhs=e_t[:, ts(k, 512)],
                                start=(kt == 0),
                                stop=(kt == 15),
                                tile_position=(0, po_av + 32 * k),
                                skip_group_check=True,
                            )
                            nc.tensor.matmul(
                                acc[ds(po_rs + 32 * k, 32), :],
                                lhsT=ones_sb,
                                rhs=e_t[:, ts(k, 512)],
                                start=(kt == 0),
                                stop=(kt == 15),
                                tile_position=(0, po_rs + 32 * k),
                                skip_group_check=True,
                            )

                    for kt in range(16):
                        s_ps = spsum.tile([128, 1024], f32, tag="s")
                        for k in range(2):
                            h = 2 * g2 + k
                            i = h % 4
                            nc.tensor.matmul(
                                s_ps[:, ts(k, 512)],
                                lhsT=kT_sb[half_idx][ds(32 * i, 32), ts(kt, 128)],
                                rhs=qT_sb[half_idx][ds(32 * i, 32), ts(qc, 512)],
                                start=True,
                                stop=True,
                                tile_position=(32 * i, 0),
                            )
                        e_sb = ep.tile([128, 1024], bf16, tag="e")
                        e_tiles[kt] = e_sb
                        nc.scalar.activation(e_sb, s_ps, EXP, bias=ebias[:, :])
                        rdw0 = 2 * qc - (kt // 2) + 7
                        woff = 128 if kt % 2 == 0 else 0
                        ev = e_sb.rearrange("p (k jj f) -> p k jj f", k=2, jj=2)
                        fv = expf_view[
                            :, ds(2 * g2, 2), ds(rdw0, 2), ds(woff, 256)
                        ]
                        nc.vector.tensor_mul(ev, ev, fv)
                        if kt >= 2:
                            emit_av(kt - 2)
                    emit_av(14)
                    emit_av(15)
                    # epilogue: normalize 2 heads into oT
                    recip = rp.tile([128, 512], f32, tag="recip")
                    rep = rp.tile([128, 512], f32, tag="rep")
                    nc.vector.tensor_copy(
                        recip[ds(po_rs, 64), :], acc[ds(po_rs, 64), :]
                    )
                    nc.vector.reciprocal(
                        recip[ds(po_rs, 64), :], recip[ds(po_rs, 64), :]
                    )
                    nc.sync.dma_start(
                        out=rep[ds(po_av, 64), :], in_=recip[ds(po_rs, 64), :]
                    )
                    nc.vector.tensor_mul(
                        oT[ds(po_av, 64), half_idx, :],
                        acc[ds(po_av, 64), :],
                        rep[ds(po_av, 64), :],
                    )
            # final projections (after both qc loops; off the loop critical path)
            for qc in range(2):
                oT = oT_tiles[qc]
                for s in range(4):
                    fps = spsum.tile([128, 1024], f32, tag="s")
                    for ch in range(2):
                        nc.tensor.matmul(
                            fps[:, :C],
                            lhsT=oT[:, ch, ts(s, 128)],
                            rhs=w_sb["Wo"][:, ch, :],
                            start=(ch == 0),
                            stop=(ch == 1),
                        )
                    stage = stp.tile([128, C], f32, tag="stage")
                    nc.vector.tensor_copy(stage, fps[:, :C])
                    nc.sync.dma_start(
                        out=out_d[ds(512 * qc + 128 * s, 128), :], in_=stage
                    )

    nc.compile()
    return nc


def _host_inputs(x, Wq, Wk, Wv, Wo, bias_table):
    """Build the 8 per-core input maps."""
    import ml_dtypes

    bf = ml_dtypes.bfloat16
    x = np.asarray(x, dtype=np.float32)
    T = np.asarray(bias_table, dtype=np.float32)
    xf = np.ascontiguousarray(x.reshape(B, N, C))
    idx_w = 15 + np.arange(16)[None, :] - np.arange(16)[:, None]  # [w2, w1]
    Ws = {
        "Wq": np.ascontiguousarray(np.asarray(Wq, np.float32).astype(bf)),
        "Wk": np.ascontiguousarray(np.asarray(Wk, np.float32).astype(bf)),
        "Wv": np.ascontiguousarray(np.asarray(Wv, np.float32).astype(bf)),
        "Wo": np.ascontiguousarray(np.asarray(Wo, np.float32).astype(bf)),
    }
    in_maps = []
    for c in range(8):
        b, r = c // 2, c % 2
        d1min = 4 * r
        Twin = T[:, d1min:d1min + 11]                     # [8, 11, 31, 31]
        TW = Twin[:, :, :, idx_w]                         # [8,11,31,16,16] (h,rdw,rh,w2,w1)
        TW = np.ascontiguousarray(TW.transpose(0, 1, 3, 2, 4))  # [h,rdw,w2,rh,w1]
        in_maps.append({
            "xT": np.ascontiguousarray(xf[b].T.astype(bf)),
            "xTq": np.ascontiguousarray(xf[b, QR * r:QR * (r + 1)].T.astype(bf)),
            "TW": TW,
            **Ws,
        })
    return in_maps


def kernel(x, Wq, Wk, Wv, Wo, bias_table, _results_hook=None):
    global _NC
    if _NC is None:
        _NC = _build_nc()
    from concourse.bass_utils import run_bass_kernel_spmd

    in_maps = _host_inputs(x, Wq, Wk, Wv, Wo, bias_table)
    res = run_bass_kernel_spmd(_NC, in_maps, core_ids=list(range(8)))
    if _results_hook is not None:
        _results_hook(res)
    out = np.zeros((B, N, C), dtype=np.float32)
    for c in range(8):
        b, r = c // 2, c % 2
        out[b, QR * r:QR * (r + 1)] = res.results[c]["out"]
    D, H, W = 8, 16, 16
    return out.reshape(B, D, H, W, C)
